# revision 66
# baseline (speedup 1.0000x reference)
# Trainium2 Bass kernel for single-head bidirectional attention with residual:
#   Y = softmax((X Wq + bq)(X Wk + bk)^T / sqrt(dk)) (X Wv + bv) Wo + bo;  out = X + Y
# X: (8, 2048, 1024) f32.  Data-parallel: one batch element per NeuronCore (8 cores).
#
# Per-core dataflow (fp8e4 matmul inputs where profitable, f32 PSUM
# accumulation, bf16 residual/output with a host-side f32 cast):
#   - X^T pre-transposed + fp8-cast + DMA-packed on host (each transfer one
#     contiguous 1-4KB per-partition run); weights fp8, pre-scaled by 32 (out
#     of e4m3's denormal range) and pair-packed for DoubleRow; scale factors
#     folded into the exp input scale and the ones-vector.
#   - QT/KT (bf16, d_k x seq) and VT via weight-stationary fp8 DoubleRow
#     matmuls; per-partition biases added on DVE; V natural (seq x d_v, fp8)
#     via PE transposes of VT.
#   - Attention runs as ONE pipelined pair stream across all q-chunks: the
#     exp on ACT (~1.15us per (128,2,qc) t-block-pair ACTIVATE) is the pacing
#     engine, so S^T matmuls (st = K_tb @ QT, bf16) run two pairs ahead, and
#     the next chunk's S pairs are emitted before the current chunk's
#     drain/output - ACT never idles at chunk boundaries. A persistent et
#     tile keeps cross-chunk deps range-based. Late projection slabs are
#     spread one-projection-at-a-time through the early stream steps, each
#     just before its first consumer, paced to the X^T piece arrivals.
#   - d = ones^T E and U = V^T E (unnormalized H^T) via fp8 DoubleRow
#     pair-matmuls; 1/d deferred to the output phase as a per-partition
#     scale (tiny PE transposes put d on q-partitions).
#   - Y = H^T_qb^T @ Wo (bf16); out = Y*recip_d + (X + bo) fused in one DVE
#     scalar_tensor_tensor per 512-slice (X + bo pre-folded on host, bf16).
#   - Startup: ACT table-load + 24 PE clock-warm dummy matmuls emitted before
#     any HWDGE trigger instruction (each trigger occupies its queue
#     ~0.7-2us); wq ships ahead of the wk/wv pack; q tapered 3x512 + 2x256.
# PSUM: 4 banks S-pairs (2x double-buffered) + 1 U + 1 d/rt (shared,
# sequential lifetimes) + 2 Y (double-buffered) = 8.
import numpy as np
from contextlib import ExitStack

import concourse.bass as bass
import concourse.mybir as mybir
import concourse.tile as tile
from concourse.bass_utils import run_bass_kernel_spmd
from concourse.bass import _add_dep_helper
from concourse.masks import make_identity

F32 = mybir.dt.float32
BF16 = mybir.dt.bfloat16
F8 = mybir.dt.float8e4
DR = mybir.MatmulPerfMode.DoubleRow
AF = mybir.ActivationFunctionType
OP = mybir.AluOpType

S, E, DK = 2048, 1024, 128
P = 128
N_CORES = 8
# fp8 weight pre-scale: W values (~0.02 std) sit in e4m3's denormal range,
# so weights ship as 32*W; the 32*32 from Q'K' and 1/sqrt(dk) fold into the
# exp input scale, the V-side 32 folds into the ones-vector (32.0) so
# rt = 1/(32 d) normalizes U' = 32 U.
WSC = 32.0


def build(S=S, E=E, DK=DK, QC=512):
    EB = E // P            # e blocks (contraction blocks for projections)
    TB = S // P            # t blocks (key/value row blocks)
    NQ = S // QC           # q chunks
    QB = QC // P           # q blocks per chunk
    JW = min(512, S)       # psum free-dim slice width for QT/KT
    YW = min(512, E)       # psum free-dim slice width for Y

    EB2 = EB // 2
    nc = bass.Bass()
    # residual + output ship as bf16: ~3e-3 elementwise rounding on an
    # output dominated by X (|Y| ~ 0.003|X|) stays far under the 2e-2 gate,
    # and it halves the two 8MB HBM streams.
    xres = nc.declare_dram_parameter("xres", [S, E], BF16, isOutput=False)
    # X^T ships pre-packed per DMA transfer (host-side): each transfer is one
    # contiguous per-partition run (1-4KB), keeping the HBM-side descriptor
    # runs well above the 512B line-rate threshold (512B runs measured only
    # ~200 GB/s; 2KB runs ~350 GB/s).
    xt = nc.declare_dram_parameter("xt", [P, EB * S], F8, isOutput=False)
    # QKV weights and biases each ship as ONE packed transfer: every HWDGE
    # trigger instruction occupies its engine queue for ~0.7-2us, and extra
    # triggers ahead of the xt pieces delay the whole startup
    wqkv = nc.declare_dram_parameter(
        "wqkv", [P, 3, EB2, 2, DK], F8, isOutput=False)
    bqkv = nc.declare_dram_parameter("bqkv", [DK, 3], F32, isOutput=False)
    wo = nc.declare_dram_parameter("wo", [DK, E], BF16, isOutput=False)
    out = nc.declare_dram_parameter("out", [S, E], BF16, isOutput=True)

    with ExitStack() as ctx:
        tc = ctx.enter_context(tile.TileContext(nc))
        const = ctx.enter_context(tc.tile_pool(name="const", bufs=1))
        ps_mm = ctx.enter_context(tc.tile_pool(name="ps_mm", bufs=2, space="PSUM"))
        ps_acc = ctx.enter_context(tc.tile_pool(name="ps_acc", bufs=1, space="PSUM"))
        # PSUM budget: 4 ps_mm + 2 ps_acc + 2 ps_y = 8 banks (d and rt share
        # one bank, their lifetimes are sequential within a chunk; ps_y is
        # double-buffered so Y matmuls overlap the output STT drains).
        ps_y = ctx.enter_context(tc.tile_pool(name="ps_y", bufs=2, space="PSUM"))
        xr_pool = ctx.enter_context(tc.tile_pool(name="xr", bufs=8))
        o_pool = ctx.enter_context(tc.tile_pool(name="o", bufs=8))
        work = ctx.enter_context(tc.tile_pool(name="work", bufs=1))
        small = ctx.enter_context(tc.tile_pool(name="small", bufs=2))

        # ---- persistent SBUF tensors ----
        w_sb = const.tile([P, 3, EB2, 2, DK], F8)
        b_all = const.tile([DK, 3], F32)
        xt_sb = const.tile([P, EB, S], F8)
        wo_sb = const.tile([DK, E], BF16)


        # Constants + ACT/PE warm-up FIRST: they must sit ahead of the DMA
        # trigger instructions in the scalar/PE queues, or the ACT table
        # load and clock warm-up happen mid-startup instead of during the
        # dead preamble window.
        ones_sb = const.tile([P, 2, 16], F8)
        nc.gpsimd.memset(ones_sb[:], WSC)
        idone = const.tile([1, 1], F32)
        nc.gpsimd.memset(idone[:], 1.0)
        ident = const.tile([P, P], BF16)
        make_identity(nc, ident[:])
        zero_b = const.tile([P, 1], F32)
        nc.gpsimd.memset(zero_b[:], 0.0)
        warm = const.tile([P, 1], F32)
        nc.scalar.activation(warm[:], zero_b[:], AF.Identity, bias=zero_b[:])
        nc.scalar.activation(warm[:], warm[:], AF.Exp, bias=zero_b[:])
        # dummy matmuls start the PE HAM clock ramp (1.2 -> 2.4 GHz after
        # ~3.4us of activity) and keep it from re-throttling (3.4us idle
        # window) while the first X^T piece + wq stream in (~4us)
        warm_ps = ps_y.tile([P, P], F32, tag="y")
        for _ in range(24):
            nc.tensor.matmul(
                warm_ps[:], ident[:], ident[:], start=True, stop=True
            )

        # ---- input DMAs: 8 triggers total across three queues ----
        xt_dmas = []
        # transfer boundaries chosen so each lands just before the PE needs
        # it (projections re-tiled to match); small first piece so the first
        # projection starts ASAP
        # element offsets into the packed xt buffer, keyed by piece start
        # (pack order on the host: 0, 256, 768, 1280, 1792)
        xt_offs = {0: 0, 256: 2048, 768: 6144, 1280: 10240, 1792: 14336}

        def xt_load(eng, t0, w):
            n = EB * w
            off = xt_offs[t0]
            src = xt[:, off:off + n].rearrange("p (b t) -> p b t", b=EB)
            xt_dmas.append(eng.dma_start(
                xt_sb[:, :, t0:t0 + w], src,
            ))

        # wq ships separately ahead of wk/wv so the first projection's
        # LDWEIGHTS doesn't wait on the full 3-weight pack
        xt_load(nc.scalar, 0, 256)
        nc.sync.dma_start(w_sb[:, 0:1], wqkv[:, 0:1])
        nc.scalar.dma_start(b_all[:], bqkv[:])
        nc.sync.dma_start(w_sb[:, 1:3], wqkv[:, 1:3])
        xt_load(nc.scalar, 768, 512)
        xt_load(nc.sync, 256, 512)
        xt_load(nc.scalar, 1792, 256)
        xt_load(nc.sync, 1280, 512)
        nc.scalar.dma_start(wo_sb[:], wo[:])

        qt_sb = const.tile([P, S], BF16)
        kt_sb = const.tile([P, S], BF16)
        v_sb = const.tile([P, TB, DK], F8)

        # ---- projections: per column slab QT-j / KT-j / VT-j (weight-
        # stationary fp8 DoubleRow), then PE-transpose VT-j's t-blocks into
        # natural (t x d_v) fp8 layout for the U matmul ----
        vt_sb = const.tile([P, S], BF16)

        PROJ_DST = (qt_sb, kt_sb, vt_sb)

        def proj_one(i, t0, jw):
            # proj psum tiles come from ps_y (1-bank tiles), NOT ps_mm: the
            # in-stream slabs would otherwise interleave with the S-pair ring
            # and block the PE on unrelated exp drains
            dst = PROJ_DST[i]
            ps = ps_y.tile([P, jw], F32, tag="y", name="proj_ps")
            for g in range(EB2):
                nc.tensor.matmul(
                    ps[:],
                    w_sb[:, i, g, :, :],
                    xt_sb[:, 2 * g:2 * g + 2, t0:t0 + jw],
                    start=(g == 0),
                    stop=(g == EB2 - 1),
                    perf_mode=DR,
                )
            nc.vector.tensor_scalar_add(
                dst[:, t0:t0 + jw], ps[:], b_all[:, i:i + 1],
            )

        def trans_slab(t0, jw):
            gsz = jw // P
            tb0 = t0 // P
            tps = ps_y.tile([P, gsz, P], BF16, tag="y")
            for i in range(gsz):
                tb = tb0 + i
                nc.tensor.transpose(
                    tps[:, i, :], vt_sb[:, tb * P:(tb + 1) * P], ident[:]
                )
            nc.vector.tensor_copy(v_sb[:, tb0:tb0 + gsz, :], tps[:])

        def proj_slab(t0, jw):
            for i in range(3):
                proj_one(i, t0, jw)
            trans_slab(t0, jw)

        # ---- attention: one continuous pair stream across all chunks ----
        # The exp on ACT is the pacing engine (~1.15us/pair vs ~0.85us of PE
        # work), so the S matmuls run a 2-pair software pipeline ahead of the
        # exps; at chunk boundaries the next chunk's first S pairs are
        # emitted BEFORE the current chunk's drain/output, keeping ACT
        # saturated end to end. The remaining projection slabs and each
        # chunk's output phase slot into the PE's spare capacity.
        esc = float(1.0 / (WSC * WSC * np.sqrt(DK)))

        # Mild taper only: every extra chunk costs ~0.5us/pair of fixed PE
        # work (stationary reloads for S and U are per-pair regardless of
        # chunk width), so deep tapers lose more mid-stream than they save
        # in tail drain. The FIRST chunk is 256-wide so its S matmuls only
        # need the head projection slab (qt[0:256]) - the exp stream starts
        # ~2us earlier, while the X^T tail is still streaming in.
        chunks = [(0, 256), (256, 512), (768, 512), (1280, 512), (1792, 256)]
        NPAIR = TB // 2
        # et persists across chunks: cross-chunk deps stay range-based
        # (different t-block slots), so exp(c+1, g0) never waits on the
        # whole-tile WAR a per-chunk tile would impose.
        et = const.tile([P, TB, QC], F8)
        pair_stp = {}
        chunk_acc = {}

        def s_pair(k):
            c, g = divmod(k, NPAIR)
            q0, qc = chunks[c]
            stp = ps_mm.tile([P, 2, qc], F32, tag="mm", name="stp")
            for h in range(2):
                tb = 2 * g + h
                nc.tensor.matmul(
                    stp[:, h, :],
                    kt_sb[:, tb * P:(tb + 1) * P],
                    qt_sb[:, q0:q0 + qc],
                    start=True,
                    stop=True,
                )
            pair_stp[k] = stp

        def exp_act(k):
            c, g = divmod(k, NPAIR)
            q0, qc = chunks[c]
            if g == 0:
                u_ps = ps_acc.tile([P, qc], F32, tag="u", name="u_ps")
                d_ps = ps_acc.tile([1, qc], F32, tag="d", name="d_ps")
                chunk_acc[c] = (u_ps, d_ps)
            stp = pair_stp.pop(k)
            nc.scalar.activation(
                et[:, 2 * g:2 * g + 2, 0:qc], stp[:], AF.Exp,
                bias=zero_b[:], scale=esc,
            )

        def du(k):
            c, g = divmod(k, NPAIR)
            q0, qc = chunks[c]
            u_ps, d_ps = chunk_acc[c]
            nc.tensor.matmul(
                u_ps[:],
                v_sb[:, 2 * g:2 * g + 2, :],
                et[:, 2 * g:2 * g + 2, 0:qc],
                start=(g == 0), stop=(g == NPAIR - 1),
                perf_mode=DR,
            )
            # softmax denominator rides the PE too: a DoubleRow ones-
            # matmul (M=1, trivial weight load) per fp8 t-block pair
            nc.tensor.matmul(
                d_ps[:],
                ones_sb[:, :, 0:1],
                et[:, 2 * g:2 * g + 2, 0:qc],
                start=(g == 0), stop=(g == NPAIR - 1),
                perf_mode=DR,
            )

        def attn_finish(c):
            q0, qc = chunks[c]
            u_ps, d_ps = chunk_acc.pop(c)
            qbs = qc // P
            ht = small.tile([P, qc], BF16, tag="ht")
            nc.vector.tensor_copy(ht[:], u_ps[:])
            # d (1, QC) -> SBUF -> transpose 128-slices onto partitions ->
            # reciprocal in the wide layout (on DVE: ACT is the attention
            # phase's busiest engine, keep it exp-only)
            dr = small.tile([1, qc], F32, tag="dr")
            nc.vector.tensor_copy(dr[:], d_ps[:])
            # reuse the d bank (bufs=1, same tag): rt's write naturally waits
            # for the dr copy, which is the last reader of d_ps
            rt_ps = ps_acc.tile([P, qbs], F32, tag="d")
            for qb in range(qbs):
                nc.tensor.matmul(
                    rt_ps[:, qb:qb + 1],
                    dr[0:1, qb * P:(qb + 1) * P],
                    idone[:],
                    is_transpose=True,
                )
            rt = small.tile([P, qbs], F32, tag="rt_sb")
            nc.vector.reciprocal(rt[:], rt_ps[:])

            # ---- phase 3: output projection + residual for this chunk ----
            xr = None
            for qb in range(qbs):
                row0 = q0 + qb * P
                if qb % 2 == 0:
                    # bf16 residual, 2 q-blocks per SWDGE transfer (fewer
                    # ~0.7us trigger instructions on the POOL queue)
                    nb = min(2, qbs - qb)
                    xr = xr_pool.tile([P, nb, E], BF16, tag="xr")
                    xr_dma = nc.gpsimd.dma_start(
                        xr[:],
                        xres[row0:row0 + nb * P, :].rearrange(
                            "(b p) e -> p b e", p=P),
                    )
                    # Keep the residual stream out of the startup DMA burst:
                    # the SDMA engines round-robin at packet granularity, so
                    # without this edge the first xt block completes only
                    # after ~all concurrently-issued bytes.
                    _add_dep_helper(
                        xr_dma.ins, xt_dmas[-1].ins, sync=True,
                        reason="xres loads deferred behind xt",
                    )
                o_sb = o_pool.tile([P, E], BF16, tag="o")
                tail = q0 >= 1536
                for j in range(E // YW):
                    y_ps = ps_y.tile([P, YW], F32, tag="y")
                    nc.tensor.matmul(
                        y_ps[:],
                        ht[:, qb * P:(qb + 1) * P],
                        wo_sb[:, j * YW:(j + 1) * YW],
                        start=True,
                        stop=True,
                    )
                    nc.vector.scalar_tensor_tensor(
                        o_sb[:, j * YW:(j + 1) * YW],
                        y_ps[:],
                        rt[:, qb:qb + 1],
                        xr[:, qb % 2, j * YW:(j + 1) * YW],
                        OP.mult,
                        OP.add,
                    )
                    if tail:
                        # tail chunks: store each 512-slice as soon as its
                        # STT lands, alternating queues - halves the final
                        # drain and overlaps it with the remaining compute
                        eng = nc.scalar if (2 * qb + j) % 2 else nc.sync
                        eng.dma_start(
                            out[row0:row0 + P, j * YW:(j + 1) * YW],
                            o_sb[:, j * YW:(j + 1) * YW],
                        )
                if not tail:
                    nc.sync.dma_start(out[row0:row0 + P, :], o_sb[:])

        # ---- emission order: two head slabs, then the flat pair stream
        # with a 2-pair S pipeline. The remaining projection work is split
        # into single-projection units spread across early stream steps
        # (each just before its first consumer), so the PE overload per step
        # stays small and the ACT exp stream never starves. ----
        NK = NPAIR * len(chunks)
        proj_slab(0, 256)
        proj_slab(256, 512)
        s_pair(0)
        s_pair(1)
        late_units = {
            1: [lambda: proj_one(1, 768, 512)],
            2: [lambda: proj_one(2, 768, 512), lambda: trans_slab(768, 512)],
            3: [lambda: proj_one(1, 1280, 512)],
            4: [lambda: proj_one(2, 1280, 512),
                lambda: trans_slab(1280, 512)],
            5: [lambda: proj_one(1, 1792, 256),
                lambda: proj_one(0, 768, 512)],
            6: [lambda: proj_one(2, 1792, 256),
                lambda: trans_slab(1792, 256)],
            8: [lambda: proj_one(0, 1280, 512)],
            10: [lambda: proj_one(0, 1792, 256)],
        }
        for k in range(NK):
            exp_act(k)
            for fn in late_units.get(k, ()):
                fn()
            # next S-pair goes ahead of this pair's d/U matmuls in the PE
            # queue (they're independent): the exp two steps out starts
            # ~0.4us earlier, which is exactly the recurring ACT stall
            if k + 2 < NK:
                s_pair(k + 2)
            du(k)
            if k % NPAIR == NPAIR - 1:
                attn_finish(k // NPAIR)

    nc.finalize()
    # walrus's queue codegen accepts at most one semaphore wait per
    # instruction ("Too many sync wait commands"); the in-compile invocations
    # of this pass leave Tile-emitted multi-waits intact, so run it once more
    # on the finalized module to split them onto InstEventSemaphore chains.
    import bass_rust
    bass_rust.generate_event_semaphores(nc)
    return nc


def make_in_maps(X, W_Q, b_Q, W_K, b_K, W_V, b_V, W_O, b_O, n_cores=N_CORES):
    import ml_dtypes
    bf16 = ml_dtypes.bfloat16
    f8 = ml_dtypes.float8_e4m3
    e, dk = W_Q.shape
    eb2 = e // P // 2
    X = np.asarray(X, np.float32)

    def pack_w(W):
        # (E, DK) -> (P, EB2, 2, DK) fp8, scaled by WSC, e = g*256 + h*128 + p
        Wp = (np.asarray(W, np.float32) * WSC).astype(f8)
        return np.ascontiguousarray(
            Wp.reshape(eb2, 2, P, dk).transpose(2, 0, 1, 3))

    shared = {
        "wqkv": np.ascontiguousarray(np.stack(
            [pack_w(W_Q), pack_w(W_K), pack_w(W_V)], axis=1)),
        "bqkv": np.ascontiguousarray(np.stack(
            [np.asarray(b, np.float32) * WSC for b in (b_Q, b_K, b_V)],
            axis=1)),
        "wo": np.ascontiguousarray(np.asarray(W_O, np.float32).astype(bf16)),
    }
    bo = np.asarray(b_O, np.float32)

    def pack_xt(xb):
        # (S, E) -> per-transfer contiguous layout (P, EB*S); order must
        # match the kernel's xt_load call sequence.
        v = xb.T.astype(f8).reshape(e // P, P, xb.shape[0]).transpose(1, 0, 2)
        parts = [
            v[:, :, 0:256], v[:, :, 256:768], v[:, :, 768:1280],
            v[:, :, 1280:1792], v[:, :, 1792:2048],
        ]
        return np.ascontiguousarray(np.concatenate(
            [p.reshape(P, -1) for p in parts], axis=1))

    in_maps = []
    for b in range(n_cores):
        xb = X[b]
        m = dict(shared)
        m["xres"] = np.ascontiguousarray((xb + bo).astype(bf16))
        m["xt"] = pack_xt(xb)
        in_maps.append(m)
    return in_maps


_CACHE = {}


def kernel(X, W_Q, b_Q, W_K, b_K, W_V, b_V, W_O, b_O):
    if "nc" not in _CACHE:
        _CACHE["nc"] = build()
    nc = _CACHE["nc"]
    in_maps = make_in_maps(X, W_Q, b_Q, W_K, b_K, W_V, b_V, W_O, b_O)
    res = run_bass_kernel_spmd(nc, in_maps, core_ids=list(range(N_CORES)))
    return np.stack(
        [res.results[b]["out"] for b in range(N_CORES)], axis=0
    ).astype(np.float32)

